# revision 10
# baseline (speedup 1.0000x reference)
"""MoE top-1 routing kernel for Trainium2 (8 NeuronCores, expert-F-sharded).

Model (E=8, D=512, F=2048, N=4096):
    logits = x @ Wg + bg; e = argmax(logits)
    y[i] = relu(x[i] @ W1[e] + b1[e]) @ W2[e] + b2[e]

Strategy (v4 — bf16 + quarter-F expert sharding):
- Host computes the gate (f64 matmul + argmax) and routes tokens.
- Each expert's FFN is split into 4 F-quarters (Fs=512). Experts are paired
  (adjacent in sorted-count order) into 4 "slots"; slot s appears on every
  core with the same compile-time token width W_s = max count over the
  slot's two experts. Core j, slot s holds (expert = pair[s][j//4],
  quarter q = j%4) and processes ALL of that expert's tokens against its
  F-quarter. PE work per core = sum_s W_s*Fs/16 cycles — near
  count-independent, so expert imbalance no longer pads every core.
- Everything on the wire is bf16 (halves HBM traffic, enables FWL fast
  weight load; rel-err ~4e-3 vs the 2e-2 gate). PSUM accumulates fp32;
  b1 is applied in the Relu, b2 is added only by the q==0 cell (zeros
  elsewhere), partial y's are summed on the host in fp32.
- y is chunk-major so every output DMA is contiguous per partition.
- Input DMA pieces are issued in exact first-consumption order of the
  software-pipelined emission (st1 of chunk i+1 between st1 and st2 of
  chunk i), with extra-fine first pieces so the first matmul fires early.
- The smallest chunk runs last (short drain tail); its PSUM->SBUF copies
  alternate ACT/DVE and its per-d output DMAs spread across engines.
- A dummy-matmul burst warms the PE clock (HAM) during the DMA head.
"""

import sys

sys.path.insert(0, "/opt/trn_rl_repo")

import numpy as np
import ml_dtypes

BF16 = ml_dtypes.bfloat16
E, D, F, N_CORES = 8, 512, 2048, 8
KD = D // 128      # 4 contraction tiles (stage1) == output d tiles (stage2)
FS = 512           # F-columns per slot (quarter of F)
KQ = FS // 128     # 4
NSLOT = 4

_cache: dict = {}


def _chunks_of(w: int) -> list[int]:
    # split width into <=512-col chunks (PSUM bank limit), evenly
    if w <= 512:
        return [w]
    n = -(-w // 512)
    base = (-(-w // n) + 15) // 16 * 16
    out, rem = [], w
    while rem > 0:
        c = min(base, rem)
        out.append(c)
        rem -= c
    return out


def _chunk_list(widths):
    """Chunk schedule: (slot, lo, cw, xoff, yoff) in execution order; the
    smallest chunk is moved to the end to shorten the drain tail."""
    ch = []
    for s, w in enumerate(widths):
        lo = 0
        for cw in _chunks_of(w):
            ch.append([s, lo, cw])
            lo += cw
    k = min(range(len(ch)), key=lambda i: (ch[i][2], -i))
    ch.append(ch.pop(k))
    off = 0
    out = []
    for s, lo, cw in ch:
        out.append((s, lo, cw, off, off))  # x and y share chunk-major offsets
        off += KD * cw
    return out, off


def _build(widths: tuple[int, ...]):
    import concourse.tile as tile
    import concourse.mybir as mybir
    from concourse import bacc

    f32 = mybir.dt.float32
    bf16 = mybir.dt.bfloat16
    Relu = mybir.ActivationFunctionType.Relu
    Ident = mybir.ActivationFunctionType.Identity

    nc = bacc.Bacc("TRN2", target_bir_lowering=False, debug=False)

    CH, xtot = _chunk_list(widths)
    n_ch = len(CH)

    # Layouts (all [128, *]):
    #   w[p, s*4096 + f*512 + ko*128 + c]        = W1[e][128*ko+p, 512*q + 128*f + c]
    #   w[p, s*4096 + 2048 + d*512 + fo*128 + c] = W2[e][512*q + 128*fo + p, 128*d + c]
    #   x[p, xoff + ko*cw + c]                   = x[tok_{lo+c}, 128*ko + p]
    #   b[p, s*8+f] = b1[e][512*q+128*f+p];  b[p, s*8+4+d] = b2[e][128*d+p] (q==0 else 0)
    #   y[p, yoff + d*cw + c]                    = partial y[tok_{lo+c}, 128*d+p]
    w_d = nc.dram_tensor("w", [128, NSLOT * 4096], bf16, kind="ExternalInput").ap()
    x_d = nc.dram_tensor("x", [128, xtot], bf16, kind="ExternalInput").ap()
    b_d = nc.dram_tensor("b", [128, NSLOT * 8], f32, kind="ExternalInput").ap()
    y_d = nc.dram_tensor("y", [128, xtot], bf16, kind="ExternalOutput").ap()

    # Emission plan: st1(i+1) between st1(i) and st2(i)
    plan = [("st1", 0)]
    for i in range(n_ch):
        if i + 1 < n_ch:
            plan.append(("st1", i + 1))
        plan.append(("st2", i))

    # Input DMA pieces: each dma_start costs ~0.6us on the issuing sequencer
    # and only ~8 can be outstanding per DGE, so keep pieces few and large,
    # and split across two engines: weights on sync (HWDGE), x on gpsimd
    # (SWDGE — its own sem lanes). Order within each stream = consumption
    # order of the software-pipelined emission.
    wpieces, xpieces, seen = [], [], set()

    def needw(lo, hi):
        if lo not in seen:
            seen.add(lo)
            wpieces.append((lo, hi))

    for i, (op, ci) in enumerate(plan):
        s, lo, cw, xoff, yoff = CH[ci]
        wb = s * 4096
        if op == "st1":
            if ci == 0:
                needw(wb, wb + 1024)        # f0-1 for an early start
                needw(wb + 1024, wb + 2048)
            else:
                needw(wb, wb + 2048)
        else:
            needw(wb + 2048, wb + 4096)

    # x: first chunk's ko0 alone (first matmul fires early), then the rest of
    # chunk 0 merged with chunk 1 (chunk-major layout is contiguous), then
    # one piece per chunk.
    c0 = CH[0]
    xpieces.append((c0[3], c0[3] + c0[2]))
    xpieces.append((c0[3] + c0[2], CH[1][3] + KD * CH[1][2]))
    for i in range(2, n_ch):
        s, lo, cw, xoff, yoff = CH[i]
        xpieces.append((xoff, xoff + KD * cw))

    with tile.TileContext(nc) as tc:
        with tc.tile_pool(name="wp", bufs=1) as wp, \
             tc.tile_pool(name="hp", bufs=2) as hp, \
             tc.tile_pool(name="yp", bufs=2) as yp, \
             tc.tile_pool(name="scr", bufs=1) as scr, \
             tc.tile_pool(name="pp", bufs=3, space="PSUM") as pp:

            # --- PE warm-up: dummy matmuls during the DMA head (HAM ramp).
            wrm = scr.tile([128, 256], bf16, name="wrm")
            nc.vector.memset(wrm[:], 0.0)
            wps = pp.tile([128, 256], f32, name="wps", tag="wps", bufs=1)
            for _ in range(14):
                nc.tensor.matmul(wps[:], wrm[:, :128], wrm[:], start=True, stop=True)

            # --- DMA issue: weights on sync (HWDGE), b + x on gpsimd (SWDGE)
            bis = wp.tile([128, NSLOT * 8], f32, name="bis")
            nc.gpsimd.dma_start(bis[:], b_d[:])

            wt = wp.tile([128, NSLOT * 4096], bf16, name="wt")
            xt = wp.tile([128, xtot], bf16, name="xt")
            for lo, hi in wpieces:
                nc.sync.dma_start(wt[:, lo:hi], w_d[:, lo:hi])
            for lo, hi in xpieces:
                nc.gpsimd.dma_start(xt[:, lo:hi], x_d[:, lo:hi])

            # --- compute ---
            hs = {}

            def st1(ci):
                s, lo, cw, xoff, yoff = CH[ci]
                for f in range(KQ):
                    p1 = pp.tile([128, 512], f32, name=f"p1_{ci}_{f}", tag="p1")
                    for ko in range(KD):
                        lhsT = wt[:, s * 4096 + f * 512 + ko * 128:
                                  s * 4096 + f * 512 + ko * 128 + 128]
                        rhs = xt[:, xoff + ko * cw: xoff + (ko + 1) * cw]
                        nc.tensor.matmul(p1[:, :cw], lhsT, rhs,
                                         start=(ko == 0), stop=(ko == KD - 1))
                    h = hp.tile([128, 512], bf16, name=f"h{ci}_{f}", tag=f"h{f}")
                    nc.scalar.activation(h[:, :cw], p1[:, :cw], Relu,
                                         bias=bis[:, s * 8 + f: s * 8 + f + 1])
                    hs[(ci, f)] = h

            def st2(ci, last):
                s, lo, cw, xoff, yoff = CH[ci]
                ys = yp.tile([128, KD * 512], bf16, name=f"ys{ci}", tag="ys")
                for d in range(KD):
                    p2 = pp.tile([128, 512], f32, name=f"p2_{ci}_{d}",
                                 tag=f"p2_{d}", bufs=1)
                    for fo in range(KQ):
                        lhsT = wt[:, s * 4096 + 2048 + d * 512 + fo * 128:
                                  s * 4096 + 2048 + d * 512 + fo * 128 + 128]
                        nc.tensor.matmul(p2[:, :cw], lhsT, hs[(ci, fo)][:, :cw],
                                         start=(fo == 0), stop=(fo == KQ - 1))
                    bcol = bis[:, s * 8 + 4 + d: s * 8 + 4 + d + 1]
                    if last:
                        # alternate ACT/DVE so the drain isn't serialized on one
                        if d % 2 == 0:
                            nc.scalar.activation(ys[:, d * cw:(d + 1) * cw],
                                                 p2[:, :cw], Ident, bias=bcol)
                        else:
                            nc.vector.tensor_scalar_add(ys[:, d * cw:(d + 1) * cw],
                                                        p2[:, :cw], bcol)
                        eng = [nc.gpsimd, nc.scalar, nc.gpsimd, nc.sync][d]
                        eng.dma_start(y_d[:, yoff + d * cw: yoff + (d + 1) * cw],
                                      ys[:, d * cw:(d + 1) * cw])
                    else:
                        nc.vector.tensor_scalar_add(ys[:, d * cw:(d + 1) * cw],
                                                    p2[:, :cw], bcol)
                if not last:
                    nc.scalar.dma_start(y_d[:, yoff: yoff + KD * cw],
                                        ys[:, :KD * cw])

            for op, ci in plan:
                if op == "st1":
                    st1(ci)
                else:
                    st2(ci, last=(ci == n_ch - 1))

    nc.compile()
    return nc


def _get_nc(widths: tuple[int, ...]):
    if widths not in _cache:
        _cache[widths] = _build(widths)
    return _cache[widths]


def _plan(counts):
    """Pair adjacent experts in sorted order into NSLOT slots (minimizes
    sum of per-slot maxima); return (pairs, widths)."""
    order = np.argsort(-counts, kind="stable")
    pairs = [(int(order[2 * s]), int(order[2 * s + 1])) for s in range(NSLOT)]
    widths = tuple(
        (max(int(counts[a]), int(counts[b]), 16) + 15) // 16 * 16
        for a, b in pairs)
    return pairs, widths


def _pack_inputs(x, W1, b1, W2, b2, order, starts, pairs, widths):
    """Build per-core in_maps. Core j, slot s: expert pair[s][j//4], quarter j%4."""
    CH, xtot = _chunk_list(widths)
    xbf = x.astype(BF16)
    toks_of = [order[starts[e]:starts[e + 1]] for e in range(E)]
    in_maps = []
    for j in range(N_CORES):
        q = j % 4
        wcols = np.empty((128, NSLOT * 4096), BF16)
        bcols = np.zeros((128, NSLOT * 8), np.float32)
        xcols = np.zeros((128, xtot), BF16)
        xe_cache = {}
        for s in range(NSLOT):
            e = pairs[s][0] if j < 4 else pairs[s][1]
            # w1 (f-major): [p, f*512 + ko*128 + c]
            w1s = W1[e][:, FS * q: FS * (q + 1)]               # [D, Fs]
            wcols[:, s * 4096: s * 4096 + 2048] = \
                w1s.reshape(KD, 128, KQ, 128).transpose(1, 2, 0, 3).reshape(128, KD * FS)
            # w2 (d-major): [p, d*512 + fo*128 + c]
            w2s = W2[e][FS * q: FS * (q + 1), :]               # [Fs, D]
            wcols[:, s * 4096 + 2048: s * 4096 + 4096] = \
                w2s.reshape(KQ, 128, KD, 128).transpose(1, 2, 0, 3).reshape(128, KQ * D)
            bcols[:, s * 8: s * 8 + KQ] = b1[e][FS * q: FS * (q + 1)].reshape(KQ, 128).T
            if q == 0:
                bcols[:, s * 8 + 4: s * 8 + 8] = b2[e].reshape(KD, 128).T
            toks = toks_of[e]
            xe = np.zeros((widths[s], D), BF16)
            xe[:len(toks)] = xbf[toks]
            xe_cache[s] = xe.T                                  # [D, W]
        for s, lo, cw, xoff, yoff in CH:
            xcols[:, xoff: xoff + KD * cw] = \
                xe_cache[s][:, lo:lo + cw].reshape(KD, 128, cw) \
                .transpose(1, 0, 2).reshape(128, KD * cw)
        in_maps.append({
            "w": np.ascontiguousarray(wcols),
            "x": np.ascontiguousarray(xcols),
            "b": bcols,
        })
    return in_maps, toks_of


def kernel(x, Wg, bg, W1, b1, W2, b2):
    from concourse.bass_utils import run_bass_kernel_spmd

    x = np.asarray(x, dtype=np.float32)
    n_tok = x.shape[0]

    # host gate in f64: the mathematically-true argmax
    logits = x.astype(np.float64) @ np.asarray(Wg, np.float64) + np.asarray(bg, np.float64)
    idx = logits.argmax(1)

    counts = np.bincount(idx, minlength=E)
    order = np.argsort(idx, kind="stable")
    starts = np.zeros(E + 1, np.int64)
    starts[1:] = np.cumsum(counts)

    pairs, widths = _plan(counts)

    W1 = np.asarray(W1, np.float32)
    W2 = np.asarray(W2, np.float32)
    b1 = np.asarray(b1, np.float32)
    b2 = np.asarray(b2, np.float32)

    in_maps, toks_of = _pack_inputs(x, W1, b1, W2, b2, order, starts, pairs, widths)
    nc = _get_nc(widths)
    res = run_bass_kernel_spmd(nc, in_maps, core_ids=list(range(N_CORES)))

    CH, xtot = _chunk_list(widths)
    out = np.zeros((n_tok, D), np.float32)
    for j in range(N_CORES):
        yv = res.results[j]["y"]
        for s, lo, cw, xoff, yoff in CH:
            e = pairs[s][0] if j < 4 else pairs[s][1]
            toks = toks_of[e]
            seg = toks[lo:lo + cw]
            if len(seg) == 0:
                continue
            blk = yv[:, yoff: yoff + KD * cw].astype(np.float32) \
                .reshape(128, KD, cw).transpose(2, 1, 0).reshape(cw, D)
            out[seg] += blk[:len(seg)]
    return out


# revision 12
# speedup vs baseline: 1.0654x; 1.0654x over previous
"""MoE top-1 routing kernel for Trainium2 (8 NeuronCores, expert-F-sharded).

Model (E=8, D=512, F=2048, N=4096):
    logits = x @ Wg + bg; e = argmax(logits)
    y[i] = relu(x[i] @ W1[e] + b1[e]) @ W2[e] + b2[e]

Strategy (v4 — bf16 + quarter-F expert sharding):
- Host computes the gate (f64 matmul + argmax) and routes tokens.
- Each expert's FFN is split into 4 F-quarters (Fs=512). Experts are paired
  (adjacent in sorted-count order) into 4 "slots"; slot s appears on every
  core with the same compile-time token width W_s = max count over the
  slot's two experts. Core j, slot s holds (expert = pair[s][j//4],
  quarter q = j%4) and processes ALL of that expert's tokens against its
  F-quarter. PE work per core = sum_s W_s*Fs/16 cycles — near
  count-independent, so expert imbalance no longer pads every core.
- Everything on the wire is bf16 (halves HBM traffic, enables FWL fast
  weight load; rel-err ~4e-3 vs the 2e-2 gate). PSUM accumulates fp32;
  b1 is applied in the Relu, b2 is added only by the q==0 cell (zeros
  elsewhere), partial y's are summed on the host in fp32.
- y is chunk-major so every output DMA is contiguous per partition.
- Input DMA pieces are issued in exact first-consumption order of the
  software-pipelined emission (st1 of chunk i+1 between st1 and st2 of
  chunk i), with extra-fine first pieces so the first matmul fires early.
- The smallest chunk runs last (short drain tail); its PSUM->SBUF copies
  alternate ACT/DVE and its per-d output DMAs spread across engines.
- A dummy-matmul burst warms the PE clock (HAM) during the DMA head.
"""

import sys

sys.path.insert(0, "/opt/trn_rl_repo")

import numpy as np
import ml_dtypes

BF16 = ml_dtypes.bfloat16
E, D, F, N_CORES = 8, 512, 2048, 8
KD = D // 128      # 4 contraction tiles (stage1) == output d tiles (stage2)
FS = 512           # F-columns per slot (quarter of F)
KQ = FS // 128     # 4
NSLOT = 4

_cache: dict = {}


def _chunks_of(w: int) -> list[int]:
    # split width into <=512-col chunks (PSUM bank limit), evenly
    if w <= 512:
        return [w]
    n = -(-w // 512)
    base = (-(-w // n) + 15) // 16 * 16
    out, rem = [], w
    while rem > 0:
        c = min(base, rem)
        out.append(c)
        rem -= c
    return out


def _chunk_list(widths):
    """Chunk schedule: (slot, lo, cw, xoff, yoff) in execution order; the
    smallest chunk is moved to the end to shorten the drain tail."""
    ch = []
    for s, w in enumerate(widths):
        lo = 0
        for cw in _chunks_of(w):
            ch.append([s, lo, cw])
            lo += cw
    k = min(range(len(ch)), key=lambda i: (ch[i][2], -i))
    ch.append(ch.pop(k))
    off = 0
    out = []
    for s, lo, cw in ch:
        out.append((s, lo, cw, off, off))  # x and y share chunk-major offsets
        off += KD * cw
    return out, off


def _build(widths: tuple[int, ...]):
    import concourse.tile as tile
    import concourse.mybir as mybir
    from concourse import bacc

    f32 = mybir.dt.float32
    bf16 = mybir.dt.bfloat16
    Relu = mybir.ActivationFunctionType.Relu
    Ident = mybir.ActivationFunctionType.Identity

    nc = bacc.Bacc("TRN2", target_bir_lowering=False, debug=False)

    CH, xtot = _chunk_list(widths)
    n_ch = len(CH)

    # Layouts (all [128, *]):
    #   w[p, s*4096 + f*512 + ko*128 + c]        = W1[e][128*ko+p, 512*q + 128*f + c]
    #   w[p, s*4096 + 2048 + d*512 + fo*128 + c] = W2[e][512*q + 128*fo + p, 128*d + c]
    #   x[p, xoff + ko*cw + c]                   = x[tok_{lo+c}, 128*ko + p]
    #   b[p, s*8+f] = b1[e][512*q+128*f+p];  b[p, s*8+4+d] = b2[e][128*d+p] (q==0 else 0)
    #   y[p, yoff + d*cw + c]                    = partial y[tok_{lo+c}, 128*d+p]
    w_d = nc.dram_tensor("w", [128, NSLOT * 4096], bf16, kind="ExternalInput").ap()
    x_d = nc.dram_tensor("x", [128, xtot], bf16, kind="ExternalInput").ap()
    b_d = nc.dram_tensor("b", [128, NSLOT * 8], f32, kind="ExternalInput").ap()
    y_d = nc.dram_tensor("y", [128, xtot], bf16, kind="ExternalOutput").ap()

    # Emission plan: st1(i+1) between st1(i) and st2(i)
    plan = [("st1", 0)]
    for i in range(n_ch):
        if i + 1 < n_ch:
            plan.append(("st1", i + 1))
        plan.append(("st2", i))

    # Input DMA pieces, all on sync (HWDGE) in exact first-consumption order
    # of the software-pipelined emission. ~1024-col (256KB) granularity:
    # each dma_start costs ~0.6us of sequencer issue time, so 23 pieces
    # (~14us of issue) stays just ahead of the ~358GB/s HBM drain while
    # keeping individual completions (and the first matmul) early.
    pieces, seen = [], set()

    def need(t, lo, hi):
        if (t, lo, hi) not in seen:
            seen.add((t, lo, hi))
            pieces.append((t, lo, hi))

    for op, ci in plan:
        s, lo, cw, xoff, yoff = CH[ci]
        wb = s * 4096
        if op == "st1":
            if ci == 0:
                need("w", wb, wb + 1024)
                need("x", xoff, xoff + cw)            # ko0: first MM fires early
                need("x", xoff + cw, xoff + KD * cw)
                need("w", wb + 1024, wb + 2048)
            else:
                need("w", wb, wb + 1024)
                need("x", xoff, xoff + KD * cw)
                need("w", wb + 1024, wb + 2048)
        else:
            need("w", wb + 2048, wb + 3072)
            need("w", wb + 3072, wb + 4096)

    with tile.TileContext(nc) as tc:
        with tc.tile_pool(name="wp", bufs=1) as wp, \
             tc.tile_pool(name="hp", bufs=2) as hp, \
             tc.tile_pool(name="yp", bufs=2) as yp, \
             tc.tile_pool(name="scr", bufs=1) as scr, \
             tc.tile_pool(name="pp", bufs=3, space="PSUM") as pp:

            # --- PE warm-up: dummy matmuls during the DMA head (HAM ramp).
            wrm = scr.tile([128, 256], bf16, name="wrm")
            nc.vector.memset(wrm[:], 0.0)
            wps = pp.tile([128, 256], f32, name="wps", tag="wps", bufs=1)
            for _ in range(14):
                nc.tensor.matmul(wps[:], wrm[:, :128], wrm[:], start=True, stop=True)

            # --- DMA issue (sync = HWDGE), consumption order; b on scalar ---
            bis = wp.tile([128, NSLOT * 8], f32, name="bis")
            nc.scalar.dma_start(bis[:], b_d[:])

            wt = wp.tile([128, NSLOT * 4096], bf16, name="wt")
            xt = wp.tile([128, xtot], bf16, name="xt")
            for t, lo, hi in pieces:
                if t == "w":
                    nc.sync.dma_start(wt[:, lo:hi], w_d[:, lo:hi])
                else:
                    nc.sync.dma_start(xt[:, lo:hi], x_d[:, lo:hi])

            # --- compute ---
            hs = {}

            def st1(ci):
                s, lo, cw, xoff, yoff = CH[ci]
                for f in range(KQ):
                    p1 = pp.tile([128, 512], f32, name=f"p1_{ci}_{f}", tag="p1")
                    for ko in range(KD):
                        lhsT = wt[:, s * 4096 + f * 512 + ko * 128:
                                  s * 4096 + f * 512 + ko * 128 + 128]
                        rhs = xt[:, xoff + ko * cw: xoff + (ko + 1) * cw]
                        nc.tensor.matmul(p1[:, :cw], lhsT, rhs,
                                         start=(ko == 0), stop=(ko == KD - 1))
                    h = hp.tile([128, 512], bf16, name=f"h{ci}_{f}", tag=f"h{f}")
                    nc.scalar.activation(h[:, :cw], p1[:, :cw], Relu,
                                         bias=bis[:, s * 8 + f: s * 8 + f + 1])
                    hs[(ci, f)] = h

            def st2(ci, last):
                s, lo, cw, xoff, yoff = CH[ci]
                ys = yp.tile([128, KD * 512], bf16, name=f"ys{ci}", tag="ys")
                for d in range(KD):
                    p2 = pp.tile([128, 512], f32, name=f"p2_{ci}_{d}",
                                 tag=f"p2_{d}", bufs=1)
                    for fo in range(KQ):
                        lhsT = wt[:, s * 4096 + 2048 + d * 512 + fo * 128:
                                  s * 4096 + 2048 + d * 512 + fo * 128 + 128]
                        nc.tensor.matmul(p2[:, :cw], lhsT, hs[(ci, fo)][:, :cw],
                                         start=(fo == 0), stop=(fo == KQ - 1))
                    bcol = bis[:, s * 8 + 4 + d: s * 8 + 4 + d + 1]
                    if last:
                        # alternate ACT/DVE so the drain isn't serialized on one
                        if d % 2 == 0:
                            nc.scalar.activation(ys[:, d * cw:(d + 1) * cw],
                                                 p2[:, :cw], Ident, bias=bcol)
                        else:
                            nc.vector.tensor_scalar_add(ys[:, d * cw:(d + 1) * cw],
                                                        p2[:, :cw], bcol)
                        eng = [nc.gpsimd, nc.scalar, nc.gpsimd, nc.sync][d]
                        eng.dma_start(y_d[:, yoff + d * cw: yoff + (d + 1) * cw],
                                      ys[:, d * cw:(d + 1) * cw])
                    else:
                        nc.vector.tensor_scalar_add(ys[:, d * cw:(d + 1) * cw],
                                                    p2[:, :cw], bcol)
                if not last:
                    nc.scalar.dma_start(y_d[:, yoff: yoff + KD * cw],
                                        ys[:, :KD * cw])

            for op, ci in plan:
                if op == "st1":
                    st1(ci)
                else:
                    st2(ci, last=(ci == n_ch - 1))

    nc.compile()
    return nc


def _get_nc(widths: tuple[int, ...]):
    if widths not in _cache:
        _cache[widths] = _build(widths)
    return _cache[widths]


def _plan(counts):
    """Pair adjacent experts in sorted order into NSLOT slots (minimizes
    sum of per-slot maxima); return (pairs, widths)."""
    order = np.argsort(-counts, kind="stable")
    pairs = [(int(order[2 * s]), int(order[2 * s + 1])) for s in range(NSLOT)]
    widths = tuple(
        (max(int(counts[a]), int(counts[b]), 16) + 15) // 16 * 16
        for a, b in pairs)
    return pairs, widths


def _pack_inputs(x, W1, b1, W2, b2, order, starts, pairs, widths):
    """Build per-core in_maps. Core j, slot s: expert pair[s][j//4], quarter j%4."""
    CH, xtot = _chunk_list(widths)
    xbf = x.astype(BF16)
    toks_of = [order[starts[e]:starts[e + 1]] for e in range(E)]
    in_maps = []
    for j in range(N_CORES):
        q = j % 4
        wcols = np.empty((128, NSLOT * 4096), BF16)
        bcols = np.zeros((128, NSLOT * 8), np.float32)
        xcols = np.zeros((128, xtot), BF16)
        xe_cache = {}
        for s in range(NSLOT):
            e = pairs[s][0] if j < 4 else pairs[s][1]
            # w1 (f-major): [p, f*512 + ko*128 + c]
            w1s = W1[e][:, FS * q: FS * (q + 1)]               # [D, Fs]
            wcols[:, s * 4096: s * 4096 + 2048] = \
                w1s.reshape(KD, 128, KQ, 128).transpose(1, 2, 0, 3).reshape(128, KD * FS)
            # w2 (d-major): [p, d*512 + fo*128 + c]
            w2s = W2[e][FS * q: FS * (q + 1), :]               # [Fs, D]
            wcols[:, s * 4096 + 2048: s * 4096 + 4096] = \
                w2s.reshape(KQ, 128, KD, 128).transpose(1, 2, 0, 3).reshape(128, KQ * D)
            bcols[:, s * 8: s * 8 + KQ] = b1[e][FS * q: FS * (q + 1)].reshape(KQ, 128).T
            if q == 0:
                bcols[:, s * 8 + 4: s * 8 + 8] = b2[e].reshape(KD, 128).T
            toks = toks_of[e]
            xe = np.zeros((widths[s], D), BF16)
            xe[:len(toks)] = xbf[toks]
            xe_cache[s] = xe.T                                  # [D, W]
        for s, lo, cw, xoff, yoff in CH:
            xcols[:, xoff: xoff + KD * cw] = \
                xe_cache[s][:, lo:lo + cw].reshape(KD, 128, cw) \
                .transpose(1, 0, 2).reshape(128, KD * cw)
        in_maps.append({
            "w": np.ascontiguousarray(wcols),
            "x": np.ascontiguousarray(xcols),
            "b": bcols,
        })
    return in_maps, toks_of


def kernel(x, Wg, bg, W1, b1, W2, b2):
    from concourse.bass_utils import run_bass_kernel_spmd

    x = np.asarray(x, dtype=np.float32)
    n_tok = x.shape[0]

    # host gate in f64: the mathematically-true argmax
    logits = x.astype(np.float64) @ np.asarray(Wg, np.float64) + np.asarray(bg, np.float64)
    idx = logits.argmax(1)

    counts = np.bincount(idx, minlength=E)
    order = np.argsort(idx, kind="stable")
    starts = np.zeros(E + 1, np.int64)
    starts[1:] = np.cumsum(counts)

    pairs, widths = _plan(counts)

    W1 = np.asarray(W1, np.float32)
    W2 = np.asarray(W2, np.float32)
    b1 = np.asarray(b1, np.float32)
    b2 = np.asarray(b2, np.float32)

    in_maps, toks_of = _pack_inputs(x, W1, b1, W2, b2, order, starts, pairs, widths)
    nc = _get_nc(widths)
    res = run_bass_kernel_spmd(nc, in_maps, core_ids=list(range(N_CORES)))

    CH, xtot = _chunk_list(widths)
    out = np.zeros((n_tok, D), np.float32)
    for j in range(N_CORES):
        yv = res.results[j]["y"]
        for s, lo, cw, xoff, yoff in CH:
            e = pairs[s][0] if j < 4 else pairs[s][1]
            toks = toks_of[e]
            seg = toks[lo:lo + cw]
            if len(seg) == 0:
                continue
            blk = yv[:, yoff: yoff + KD * cw].astype(np.float32) \
                .reshape(128, KD, cw).transpose(2, 1, 0).reshape(cw, D)
            out[seg] += blk[:len(seg)]
    return out
